# revision 3
# baseline (speedup 1.0000x reference)
"""Trainium2 Bass kernel for nn_HammingL2 (pairwise Hamming-weighted L2 loss).

Math: per-LUT loss = sum_{i<j} W[i,j](v_i-v_j)^2 = d.(v*v) - v^T W v with
d = rowsum(W).  Summed over all LUTs this equals  sum_ij M_ij G_ij  where
G = V^T V  (Gram over all LUTs, [256,256]) and  M = diag(d) - W.

Strategy: data-parallel over 8 NeuronCores.  Each core streams its
[8192, 256] shard of `luts` from HBM and accumulates the shard Gram
G_c = V_c^T V_c on the tensor engine (128 accumulating matmuls into two
[128,256] PSUM tiles).  Epilogue: elementwise multiply by M (precomputed
host-side from W, tiny) and row-reduce to [128,2] partials per core.
Host sums the 8*256 partials and divides by NUM_LUTS.
"""

import numpy as np

N_CORES = 8
NUM_LUTS = 65536
L = 256               # LUT_SIZE
SHARD = NUM_LUTS // N_CORES   # 8192 LUTs per core
P = 128               # partitions
CHUNKS = SHARD // P   # 64 matmul chunks per core
BLK = 8               # chunks per DMA block -> [128, 8*256] f32 = 1 MiB
NBLK = CHUNKS // BLK  # 8 DMA blocks

_CACHE = {}


def _seed_ntff_hook():
    """Make `antenv.axon_hooks` importable so run_bass_kernel_spmd(trace=True)
    can capture NTFF profiles under axon.  No-op if already present."""
    import sys
    import types

    try:
        import antenv.axon_hooks  # noqa: F401
        return
    except Exception:
        pass
    mod = types.ModuleType("antenv.axon_hooks")
    mod._hook = None

    def set_axon_ntff_profile_hook(h):
        mod._hook = h

    def get_axon_ntff_profile_hook():
        if mod._hook is None:
            try:
                from trn_agent_boot.trn_boot import _ntff_profile_via_ctypes

                mod._hook = _ntff_profile_via_ctypes("/opt/axon/libaxon_pjrt.so")
            except Exception:
                return None
        return mod._hook

    mod.set_axon_ntff_profile_hook = set_axon_ntff_profile_hook
    mod.get_axon_ntff_profile_hook = get_axon_ntff_profile_hook
    sys.modules["antenv.axon_hooks"] = mod


def _build():
    """Build + compile the per-core Bass kernel (cached)."""
    if "nc" in _CACHE:
        return _CACHE["nc"]

    import concourse.mybir as mybir
    import concourse.tile as tile
    from concourse import bacc

    f32 = mybir.dt.float32
    nc = bacc.Bacc("TRN2", target_bir_lowering=False, debug=False, num_devices=N_CORES)
    v = nc.dram_tensor("v", [SHARD, L], f32, kind="ExternalInput").ap()
    m = nc.dram_tensor("m", [L, L], f32, kind="ExternalInput").ap()
    out = nc.dram_tensor("out", [P, 2], f32, kind="ExternalOutput").ap()

    # shard rows n = (nb*BLK + c)*P + p ; free dim packs (c, j)
    v4 = v.rearrange("(nb c p) j -> nb p c j", c=BLK, p=P)
    m3 = m.rearrange("(h p) j -> h p j", p=P)

    with tile.TileContext(nc) as tc:
        with (
            tc.tile_pool(name="vpool", bufs=NBLK) as vpool,
            tc.tile_pool(name="mpool", bufs=1) as mpool,
            tc.tile_pool(name="psum", bufs=2, space="PSUM") as psum_pool,
            tc.tile_pool(name="opool", bufs=1) as opool,
        ):
            m_tiles = []
            for h in range(2):
                mt = mpool.tile([P, L], f32, tag=f"m{h}")
                nc.sync.dma_start(mt[:], m3[h])
                m_tiles.append(mt)

            g_ps = [
                psum_pool.tile([P, L], f32, tag=f"g{h}", name=f"g{h}")
                for h in range(2)
            ]

            for nb in range(NBLK):
                vt = vpool.tile([P, BLK, L], f32, tag="v")
                nc.sync.dma_start(vt[:], v4[nb])
                for c in range(BLK):
                    k = nb * BLK + c
                    rhs = vt[:, c, :]
                    for h in range(2):
                        nc.tensor.matmul(
                            g_ps[h][:],
                            vt[:, c, h * P : (h + 1) * P],
                            rhs,
                            start=(k == 0),
                            stop=(k == CHUNKS - 1),
                        )

            o_tile = opool.tile([P, 2], f32, tag="o")
            for h in range(2):
                scratch = opool.tile([P, L], f32, tag=f"scratch{h}")
                nc.vector.tensor_mul(scratch[:], g_ps[h][:], m_tiles[h][:])
                nc.vector.reduce_sum(
                    o_tile[:, h : h + 1], scratch[:], axis=mybir.AxisListType.X
                )
            nc.sync.dma_start(out, o_tile[:])

    nc.compile()
    _CACHE["nc"] = nc
    return nc


def _run(luts, W, trace=False, **trace_kwargs):
    """Shard, run on 8 cores, return (loss_scalar, BassKernelResults)."""
    _seed_ntff_hook()
    from concourse.bass_utils import run_bass_kernel_spmd

    nc = _build()

    luts = np.ascontiguousarray(np.asarray(luts, dtype=np.float32))
    W = np.asarray(W, dtype=np.float32)
    # M = diag(rowsum(W)) - W, in f64 then cast (tiny: 256x256)
    Wd = W.astype(np.float64)
    M = (np.diag(Wd.sum(axis=1)) - Wd).astype(np.float32)

    in_maps = [
        {"v": luts[i * SHARD : (i + 1) * SHARD], "m": M} for i in range(N_CORES)
    ]
    res = run_bass_kernel_spmd(
        nc, in_maps, core_ids=list(range(N_CORES)), trace=trace, **trace_kwargs
    )
    total = np.float64(0.0)
    for r in res.results:
        total += r["out"].astype(np.float64).sum()
    loss = np.asarray(total / NUM_LUTS, dtype=np.float32)
    return loss, res


def kernel(luts, W, gamma=None, **_unused):
    loss, _ = _run(luts, W, trace=False)
    return loss


if __name__ == "__main__":
    rng = np.random.default_rng(0)
    luts = rng.standard_normal((NUM_LUTS, L), dtype=np.float32)
    W = rng.random((L, L), dtype=np.float32)
    W = (W + W.T) / 2
    np.fill_diagonal(W, 0.0)
    print(kernel(luts, W))


# revision 6
# speedup vs baseline: 1.7519x; 1.7519x over previous
"""Trainium2 Bass kernel for nn_HammingL2 (pairwise Hamming-weighted L2 loss).

Math: per-LUT loss = sum_{i<j} W[i,j](v_i-v_j)^2 = d.(v*v) - v^T W v with
d = rowsum(W).  Summed over all LUTs this equals  sum_ij M_ij G_ij  where
G = V^T V  (Gram over all LUTs, [256,256]) and  M = diag(d) - W.

Strategy: data-parallel over 8 NeuronCores.  Each core streams its
[8192, 256] shard of `luts` from HBM and accumulates the shard Gram
G_c = V_c^T V_c on the tensor engine (128 accumulating matmuls into two
[128,256] PSUM tiles).  Epilogue: elementwise multiply by M (precomputed
host-side from W, tiny) and row-reduce to [128,2] partials per core.
Host sums the 8*256 partials and divides by NUM_LUTS.

The matmul operands are bitcast to float32r (single-pass fp32 matmul;
plain fp32 needs two half-speed passes = 4x cycles).
"""

import numpy as np

N_CORES = 8
NUM_LUTS = 65536
L = 256               # LUT_SIZE
SHARD = NUM_LUTS // N_CORES   # 8192 LUTs per core
P = 128               # partitions
CHUNKS = SHARD // P   # 64 matmul chunks per core
BLK = 8               # chunks per DMA block -> [128, 8*256] f32 = 1 MiB
NBLK = CHUNKS // BLK  # 8 DMA blocks

MODE = "f32r"         # matmul operand mode: "f32" | "f32r" | "bf16"

_CACHE = {}


def _seed_ntff_hook():
    """Make `antenv.axon_hooks` importable so run_bass_kernel_spmd(trace=True)
    can capture NTFF profiles under axon.  No-op if already present."""
    import sys
    import types

    try:
        import antenv.axon_hooks  # noqa: F401
        return
    except Exception:
        pass
    mod = types.ModuleType("antenv.axon_hooks")
    mod._hook = None

    def set_axon_ntff_profile_hook(h):
        mod._hook = h

    def get_axon_ntff_profile_hook():
        if mod._hook is None:
            try:
                from trn_agent_boot.trn_boot import _ntff_profile_via_ctypes

                mod._hook = _ntff_profile_via_ctypes("/opt/axon/libaxon_pjrt.so")
            except Exception:
                return None
        return mod._hook

    mod.set_axon_ntff_profile_hook = set_axon_ntff_profile_hook
    mod.get_axon_ntff_profile_hook = get_axon_ntff_profile_hook
    sys.modules["antenv.axon_hooks"] = mod


def _build(mode=None):
    """Build + compile the per-core Bass kernel (cached)."""
    mode = mode or MODE
    if mode in _CACHE:
        return _CACHE[mode]

    import concourse.mybir as mybir
    import concourse.tile as tile
    from concourse import bacc

    f32 = mybir.dt.float32
    bf16 = mybir.dt.bfloat16
    v_dt = mybir.dt.float32r if mode == "f32r" else f32
    nc = bacc.Bacc("TRN2", target_bir_lowering=False, debug=False, num_devices=N_CORES)
    v = nc.dram_tensor("v", [SHARD, L], v_dt, kind="ExternalInput").ap()
    m = nc.dram_tensor("m", [L, L], f32, kind="ExternalInput").ap()
    out = nc.dram_tensor("out", [P, 2], f32, kind="ExternalOutput").ap()

    # shard rows n = (nb*BLK + c)*P + p ; free dim packs (c, j)
    v4 = v.rearrange("(nb c p) j -> nb p c j", c=BLK, p=P)
    m3 = m.rearrange("(h p) j -> h p j", p=P)

    with tile.TileContext(nc) as tc:
        with (
            tc.tile_pool(name="vpool", bufs=NBLK) as vpool,
            tc.tile_pool(name="mpool", bufs=1) as mpool,
            tc.tile_pool(name="psum", bufs=2, space="PSUM") as psum_pool,
            tc.tile_pool(name="opool", bufs=1) as opool,
        ):
            g_ps = [
                psum_pool.tile([P, L], f32, tag=f"g{h}", name=f"g{h}")
                for h in range(2)
            ]

            for nb in range(NBLK):
                if mode == "bf16":
                    vt = vpool.tile([P, BLK, L], bf16, tag="v", name="vt")
                    nc.gpsimd.dma_start(vt[:], v4[nb])  # SWDGE cast f32->bf16
                    mm = vt
                else:
                    vt = vpool.tile([P, BLK, L], v_dt, tag="v", name="vt")
                    nc.sync.dma_start(vt[:], v4[nb])
                    mm = vt
                for c in range(BLK):
                    k = nb * BLK + c
                    rhs = mm[:, c, :]
                    for h in range(2):
                        nc.tensor.matmul(
                            g_ps[h][:],
                            mm[:, c, h * P : (h + 1) * P],
                            rhs,
                            start=(k == 0),
                            stop=(k == CHUNKS - 1),
                        )

            m_tiles = []
            for h in range(2):
                mt = mpool.tile([P, L], f32, tag=f"m{h}", name=f"m{h}")
                nc.sync.dma_start(mt[:], m3[h])
                m_tiles.append(mt)

            o_tile = opool.tile([P, 2], f32, tag="o")
            for h in range(2):
                scratch = opool.tile([P, L], f32, tag=f"scratch{h}")
                nc.vector.tensor_mul(scratch[:], g_ps[h][:], m_tiles[h][:])
                nc.vector.reduce_sum(
                    o_tile[:, h : h + 1], scratch[:], axis=mybir.AxisListType.X
                )
            nc.sync.dma_start(out, o_tile[:])

    nc.compile()
    _CACHE[mode] = nc
    return nc


def _run(luts, W, trace=False, mode=None, **trace_kwargs):
    """Shard, run on 8 cores, return (loss_scalar, BassKernelResults)."""
    _seed_ntff_hook()
    from concourse.bass_utils import run_bass_kernel_spmd

    nc = _build(mode)

    luts = np.ascontiguousarray(np.asarray(luts, dtype=np.float32))
    W = np.asarray(W, dtype=np.float32)
    # M = diag(rowsum(W)) - W, in f64 then cast (tiny: 256x256)
    Wd = W.astype(np.float64)
    M = (np.diag(Wd.sum(axis=1)) - Wd).astype(np.float32)

    in_maps = [
        {"v": luts[i * SHARD : (i + 1) * SHARD], "m": M} for i in range(N_CORES)
    ]
    res = run_bass_kernel_spmd(
        nc, in_maps, core_ids=list(range(N_CORES)), trace=trace, **trace_kwargs
    )
    total = np.float64(0.0)
    for r in res.results:
        total += r["out"].astype(np.float64).sum()
    loss = np.asarray(total / NUM_LUTS, dtype=np.float32)
    return loss, res


def kernel(luts, W, gamma=None, **_unused):
    loss, _ = _run(luts, W, trace=False)
    return loss


if __name__ == "__main__":
    rng = np.random.default_rng(0)
    luts = rng.standard_normal((NUM_LUTS, L), dtype=np.float32)
    W = rng.random((L, L), dtype=np.float32)
    W = (W + W.T) / 2
    np.fill_diagonal(W, 0.0)
    print(kernel(luts, W))


# revision 8
# speedup vs baseline: 1.7936x; 1.0238x over previous
"""Trainium2 Bass kernel for nn_HammingL2 (pairwise Hamming-weighted L2 loss).

Math: per-LUT loss = sum_{i<j} W[i,j](v_i-v_j)^2 = d.(v*v) - v^T W v with
d = rowsum(W).  Summed over all LUTs this equals  sum_ij M_ij G_ij  where
G = V^T V  (Gram over all LUTs, [256,256]) and  M = diag(d) - W.

Strategy: data-parallel over 8 NeuronCores.  Each core streams its
[8192, 256] shard of `luts` from HBM and accumulates the shard Gram
G_c = V_c^T V_c on the tensor engine (128 accumulating matmuls into two
[128,256] PSUM tiles).  Epilogue: elementwise multiply by M (precomputed
host-side from W, tiny) and row-reduce to [128,2] partials per core.
Host sums the 8*256 partials and divides by NUM_LUTS.

The matmul operands are bitcast to float32r (single-pass fp32 matmul;
plain fp32 needs two half-speed passes = 4x cycles).
"""

import numpy as np

N_CORES = 8
NUM_LUTS = 65536
L = 256               # LUT_SIZE
SHARD = NUM_LUTS // N_CORES   # 8192 LUTs per core
P = 128               # partitions
CHUNKS = SHARD // P   # 64 matmul chunks per core
BLK = 4               # chunks per DMA block -> [128, 4*256] f32 = 512 KiB
NBLK = CHUNKS // BLK  # DMA blocks

MODE = "f32r"         # matmul operand mode: "f32" | "f32r" | "bf16"

_CACHE = {}


def _seed_ntff_hook():
    """Make `antenv.axon_hooks` importable so run_bass_kernel_spmd(trace=True)
    can capture NTFF profiles under axon.  No-op if already present."""
    import sys
    import types

    try:
        import antenv.axon_hooks  # noqa: F401
        return
    except Exception:
        pass
    mod = types.ModuleType("antenv.axon_hooks")
    mod._hook = None

    def set_axon_ntff_profile_hook(h):
        mod._hook = h

    def get_axon_ntff_profile_hook():
        if mod._hook is None:
            try:
                from trn_agent_boot.trn_boot import _ntff_profile_via_ctypes

                mod._hook = _ntff_profile_via_ctypes("/opt/axon/libaxon_pjrt.so")
            except Exception:
                return None
        return mod._hook

    mod.set_axon_ntff_profile_hook = set_axon_ntff_profile_hook
    mod.get_axon_ntff_profile_hook = get_axon_ntff_profile_hook
    sys.modules["antenv.axon_hooks"] = mod


def _build(mode=None):
    """Build + compile the per-core Bass kernel (cached)."""
    mode = mode or MODE
    if mode in _CACHE:
        return _CACHE[mode]

    import concourse.mybir as mybir
    import concourse.tile as tile
    from concourse import bacc

    f32 = mybir.dt.float32
    bf16 = mybir.dt.bfloat16
    v_dt = mybir.dt.float32r if mode == "f32r" else f32
    nc = bacc.Bacc("TRN2", target_bir_lowering=False, debug=False, num_devices=N_CORES)
    v = nc.dram_tensor("v", [SHARD, L], v_dt, kind="ExternalInput").ap()
    m = nc.dram_tensor("m", [L, L], f32, kind="ExternalInput").ap()
    out = nc.dram_tensor("out", [P, 2], f32, kind="ExternalOutput").ap()

    # shard rows n = (nb*BLK + c)*P + p ; free dim packs (c, j)
    v4 = v.rearrange("(nb c p) j -> nb p c j", c=BLK, p=P)
    m3 = m.rearrange("(h p) j -> h p j", p=P)

    with tile.TileContext(nc) as tc:
        with (
            tc.tile_pool(name="vpool", bufs=NBLK) as vpool,
            tc.tile_pool(name="mpool", bufs=1) as mpool,
            tc.tile_pool(name="psum", bufs=2, space="PSUM") as psum_pool,
            tc.tile_pool(name="opool", bufs=1) as opool,
        ):
            g_ps = [
                psum_pool.tile([P, L], f32, tag=f"g{h}", name=f"g{h}")
                for h in range(2)
            ]

            for nb in range(NBLK):
                if mode == "bf16":
                    vt = vpool.tile([P, BLK, L], bf16, tag="v", name="vt")
                    nc.gpsimd.dma_start(vt[:], v4[nb])  # SWDGE cast f32->bf16
                    mm = vt
                else:
                    vt = vpool.tile([P, BLK, L], v_dt, tag="v", name="vt")
                    nc.sync.dma_start(vt[:], v4[nb])
                    mm = vt
                for c in range(BLK):
                    k = nb * BLK + c
                    rhs = mm[:, c, :]
                    for h in range(2):
                        nc.tensor.matmul(
                            g_ps[h][:],
                            mm[:, c, h * P : (h + 1) * P],
                            rhs,
                            start=(k == 0),
                            stop=(k == CHUNKS - 1),
                        )

            # m loads go on the ACT HWDGE queue so they don't delay v blocks
            m_tiles = []
            for h in range(2):
                mt = mpool.tile([P, L], f32, tag=f"m{h}", name=f"m{h}")
                nc.scalar.dma_start(mt[:], m3[h])
                m_tiles.append(mt)

            o_tile = opool.tile([P, 2], f32, tag="o")
            for h in range(2):
                scratch = opool.tile([P, L], f32, tag=f"scratch{h}")
                nc.vector.tensor_mul(scratch[:], g_ps[h][:], m_tiles[h][:])
                nc.vector.reduce_sum(
                    o_tile[:, h : h + 1], scratch[:], axis=mybir.AxisListType.X
                )
            nc.sync.dma_start(out, o_tile[:])

    nc.compile()
    _CACHE[mode] = nc
    return nc


def _run(luts, W, trace=False, mode=None, **trace_kwargs):
    """Shard, run on 8 cores, return (loss_scalar, BassKernelResults)."""
    _seed_ntff_hook()
    from concourse.bass_utils import run_bass_kernel_spmd

    nc = _build(mode)

    luts = np.ascontiguousarray(np.asarray(luts, dtype=np.float32))
    W = np.asarray(W, dtype=np.float32)
    # M = diag(rowsum(W)) - W, in f64 then cast (tiny: 256x256)
    Wd = W.astype(np.float64)
    M = (np.diag(Wd.sum(axis=1)) - Wd).astype(np.float32)

    in_maps = [
        {"v": luts[i * SHARD : (i + 1) * SHARD], "m": M} for i in range(N_CORES)
    ]
    res = run_bass_kernel_spmd(
        nc, in_maps, core_ids=list(range(N_CORES)), trace=trace, **trace_kwargs
    )
    total = np.float64(0.0)
    for r in res.results:
        total += r["out"].astype(np.float64).sum()
    loss = np.asarray(total / NUM_LUTS, dtype=np.float32)
    return loss, res


def kernel(luts, W, gamma=None, **_unused):
    loss, _ = _run(luts, W, trace=False)
    return loss


if __name__ == "__main__":
    rng = np.random.default_rng(0)
    luts = rng.standard_normal((NUM_LUTS, L), dtype=np.float32)
    W = rng.random((L, L), dtype=np.float32)
    W = (W + W.T) / 2
    np.fill_diagonal(W, 0.0)
    print(kernel(luts, W))


# revision 9
# speedup vs baseline: 1.8691x; 1.0421x over previous
"""Trainium2 Bass kernel for nn_HammingL2 (pairwise Hamming-weighted L2 loss).

Math: per-LUT loss = sum_{i<j} W[i,j](v_i-v_j)^2 = d.(v*v) - v^T W v with
d = rowsum(W).  Summed over all LUTs this equals  sum_ij M_ij G_ij  where
G = V^T V  (Gram over all LUTs, [256,256]) and  M = diag(d) - W.

Strategy: data-parallel over 8 NeuronCores.  Each core streams its
[8192, 256] shard of `luts` from HBM and accumulates the shard Gram
G_c = V_c^T V_c on the tensor engine (128 accumulating matmuls into two
[128,256] PSUM tiles, operands bitcast to float32r for single-pass fp32
matmul speed).  The raw Gram is copied to SBUF and DMA'd out; the host
computes sum(M * sum_c G_c) / NUM_LUTS (a 256x256 reduction - trivial).

The kernel is DMA-bound: 8 MiB/core of f32 reads at ~300-380 GB/s/core.
"""

import numpy as np

N_CORES = 8
NUM_LUTS = 65536
L = 256               # LUT_SIZE
SHARD = NUM_LUTS // N_CORES   # 8192 LUTs per core
P = 128               # partitions
CHUNKS = SHARD // P   # 64 matmul chunks per core

# DMA block sizes in chunks (1 chunk = [128, 256] f32 = 128 KiB).
# Mostly 4-chunk (512 KiB) blocks; tapered tail so the PE drains right
# behind the last byte.
BLOCK_SIZES = [4] * 15 + [2, 1, 1]
assert sum(BLOCK_SIZES) == CHUNKS

MODE = "f32r"         # matmul operand mode: "f32" | "f32r" | "bf16"

_CACHE = {}


def _seed_ntff_hook():
    """Make `antenv.axon_hooks` importable so run_bass_kernel_spmd(trace=True)
    can capture NTFF profiles under axon.  No-op if already present."""
    import sys
    import types

    try:
        import antenv.axon_hooks  # noqa: F401
        return
    except Exception:
        pass
    mod = types.ModuleType("antenv.axon_hooks")
    mod._hook = None

    def set_axon_ntff_profile_hook(h):
        mod._hook = h

    def get_axon_ntff_profile_hook():
        if mod._hook is None:
            try:
                from trn_agent_boot.trn_boot import _ntff_profile_via_ctypes

                mod._hook = _ntff_profile_via_ctypes("/opt/axon/libaxon_pjrt.so")
            except Exception:
                return None
        return mod._hook

    mod.set_axon_ntff_profile_hook = set_axon_ntff_profile_hook
    mod.get_axon_ntff_profile_hook = get_axon_ntff_profile_hook
    sys.modules["antenv.axon_hooks"] = mod


def _build(mode=None):
    """Build + compile the per-core Bass kernel (cached)."""
    mode = mode or MODE
    if mode in _CACHE:
        return _CACHE[mode]

    import concourse.mybir as mybir
    import concourse.tile as tile
    from concourse import bacc

    f32 = mybir.dt.float32
    bf16 = mybir.dt.bfloat16
    v_dt = mybir.dt.float32r if mode == "f32r" else f32
    nc = bacc.Bacc("TRN2", target_bir_lowering=False, debug=False, num_devices=N_CORES)
    v = nc.dram_tensor("v", [SHARD, L], v_dt, kind="ExternalInput").ap()
    out = nc.dram_tensor("out", [P, 2, L], f32, kind="ExternalOutput").ap()

    # [CHUNKS, P, L] view: chunk k covers shard rows [k*128, (k+1)*128)
    v3 = v.rearrange("(k p) j -> k p j", p=P)

    with tile.TileContext(nc) as tc:
        with (
            tc.tile_pool(name="vpool", bufs=len(BLOCK_SIZES)) as vpool,
            tc.tile_pool(name="psum", bufs=2, space="PSUM") as psum_pool,
            tc.tile_pool(name="opool", bufs=1) as opool,
        ):
            g_ps = [
                psum_pool.tile([P, L], f32, tag=f"g{h}", name=f"g{h}")
                for h in range(2)
            ]

            k = 0
            for blk in BLOCK_SIZES:
                if mode == "bf16":
                    vt = vpool.tile([P, 4, L], bf16, tag="v", name="vt")
                    nc.gpsimd.dma_start(
                        vt[:, :blk, :],
                        v3[k : k + blk].rearrange("k p j -> p k j"),
                    )
                else:
                    vt = vpool.tile([P, 4, L], v_dt, tag="v", name="vt")
                    nc.sync.dma_start(
                        vt[:, :blk, :],
                        v3[k : k + blk].rearrange("k p j -> p k j"),
                    )
                for c in range(blk):
                    rhs = vt[:, c, :]
                    for h in range(2):
                        nc.tensor.matmul(
                            g_ps[h][:],
                            vt[:, c, h * P : (h + 1) * P],
                            rhs,
                            start=(k == 0),
                            stop=(k == CHUNKS - 1),
                        )
                    k += 1

            o_tile = opool.tile([P, 2, L], f32, tag="o")
            for h in range(2):
                nc.vector.tensor_copy(o_tile[:, h, :], g_ps[h][:])
            nc.sync.dma_start(out, o_tile[:])

    nc.compile()
    _CACHE[mode] = nc
    return nc


def _run(luts, W, trace=False, mode=None, **trace_kwargs):
    """Shard, run on 8 cores, return (loss_scalar, BassKernelResults)."""
    _seed_ntff_hook()
    from concourse.bass_utils import run_bass_kernel_spmd

    nc = _build(mode)

    luts = np.ascontiguousarray(np.asarray(luts, dtype=np.float32))
    W = np.asarray(W, dtype=np.float32)

    in_maps = [{"v": luts[i * SHARD : (i + 1) * SHARD]} for i in range(N_CORES)]
    res = run_bass_kernel_spmd(
        nc, in_maps, core_ids=list(range(N_CORES)), trace=trace, **trace_kwargs
    )

    # host epilogue: loss = sum(M * G_total) / NUM_LUTS  (256x256 - trivial)
    Wd = W.astype(np.float64)
    M = np.diag(Wd.sum(axis=1)) - Wd
    G = np.zeros((L, L), dtype=np.float64)
    for r in res.results:
        g = r["out"].astype(np.float64)  # [128, 2, 256]
        G[:P] += g[:, 0, :]
        G[P:] += g[:, 1, :]
    loss = np.asarray((M * G).sum() / NUM_LUTS, dtype=np.float32)
    return loss, res


def kernel(luts, W, gamma=None, **_unused):
    loss, _ = _run(luts, W, trace=False)
    return loss


if __name__ == "__main__":
    rng = np.random.default_rng(0)
    luts = rng.standard_normal((NUM_LUTS, L), dtype=np.float32)
    W = rng.random((L, L), dtype=np.float32)
    W = (W + W.T) / 2
    np.fill_diagonal(W, 0.0)
    print(kernel(luts, W))


# revision 11
# speedup vs baseline: 2.0593x; 1.1018x over previous
"""Trainium2 Bass kernel for nn_HammingL2 (pairwise Hamming-weighted L2 loss).

Math: per-LUT loss = sum_{i<j} W[i,j](v_i-v_j)^2 = d.(v*v) - v^T W v with
d = rowsum(W).  Summed over all LUTs this equals  sum_ij M_ij G_ij  where
G = V^T V  (Gram over all LUTs, [256,256]) and  M = diag(d) - W.

Strategy: data-parallel over 8 NeuronCores.  Each core streams its
[8192, 256] shard of `luts` from HBM and accumulates the shard Gram
G_c = V_c^T V_c on the tensor engine (128 accumulating matmuls into two
[128,256] PSUM tiles, operands bitcast to float32r for single-pass fp32
matmul speed).  The raw Gram is copied to SBUF and DMA'd out; the host
computes sum(M * sum_c G_c) / NUM_LUTS (a 256x256 reduction - trivial).

The kernel is DMA-bound: 8 MiB/core of f32 reads at ~300-380 GB/s/core.
"""

import numpy as np

N_CORES = 8
NUM_LUTS = 65536
L = 256               # LUT_SIZE
SHARD = NUM_LUTS // N_CORES   # 8192 LUTs per core
P = 128               # partitions
CHUNKS = SHARD // P   # 64 matmul chunks per core

# DMA block sizes in chunks (1 chunk = 128 LUT rows = [128, 256] f32 = 128 KiB).
# Within a block of q chunks, partition p holds q CONSECUTIVE shard rows
# (r0 + p*q + c) so each partition's DMA run is q KiB contiguous -> much
# better descriptor efficiency than 1 KiB runs.  Tapered tail so the PE
# drains right behind the last byte.
BLOCK_SIZES = [4] * 15 + [2, 1, 1]
assert sum(BLOCK_SIZES) == CHUNKS

MODE = "f32r"         # matmul operand mode: "f32" | "f32r" | "bf16"

_CACHE = {}


def _seed_ntff_hook():
    """Make `antenv.axon_hooks` importable so run_bass_kernel_spmd(trace=True)
    can capture NTFF profiles under axon.  No-op if already present."""
    import sys
    import types

    try:
        import antenv.axon_hooks  # noqa: F401
        return
    except Exception:
        pass
    mod = types.ModuleType("antenv.axon_hooks")
    mod._hook = None

    def set_axon_ntff_profile_hook(h):
        mod._hook = h

    def get_axon_ntff_profile_hook():
        if mod._hook is None:
            try:
                from trn_agent_boot.trn_boot import _ntff_profile_via_ctypes

                mod._hook = _ntff_profile_via_ctypes("/opt/axon/libaxon_pjrt.so")
            except Exception:
                return None
        return mod._hook

    mod.set_axon_ntff_profile_hook = set_axon_ntff_profile_hook
    mod.get_axon_ntff_profile_hook = get_axon_ntff_profile_hook
    sys.modules["antenv.axon_hooks"] = mod


def _build(mode=None):
    """Build + compile the per-core Bass kernel (cached)."""
    mode = mode or MODE
    if mode in _CACHE:
        return _CACHE[mode]

    import concourse.mybir as mybir
    import concourse.tile as tile
    from concourse import bacc

    f32 = mybir.dt.float32
    bf16 = mybir.dt.bfloat16
    v_dt = mybir.dt.float32r if mode == "f32r" else f32
    nc = bacc.Bacc("TRN2", target_bir_lowering=False, debug=False, num_devices=N_CORES)
    v = nc.dram_tensor("v", [SHARD, L], v_dt, kind="ExternalInput").ap()
    out = nc.dram_tensor("out", [P, 2, L], f32, kind="ExternalOutput").ap()

    max_q = max(BLOCK_SIZES)

    with tile.TileContext(nc) as tc:
        with (
            tc.tile_pool(name="vpool", bufs=len(BLOCK_SIZES)) as vpool,
            tc.tile_pool(name="psum", bufs=2, space="PSUM") as psum_pool,
            tc.tile_pool(name="opool", bufs=1) as opool,
        ):
            g_ps = [
                psum_pool.tile([P, L], f32, tag=f"g{h}", name=f"g{h}")
                for h in range(2)
            ]

            k = 0
            r0 = 0
            for blk in BLOCK_SIZES:
                # partition p <- rows r0 + p*blk + [0, blk): blk KiB contiguous
                src = v[r0 : r0 + P * blk].rearrange("(p q) j -> p q j", q=blk)
                if mode == "bf16":
                    vt = vpool.tile([P, max_q, L], bf16, tag="v", name="vt")
                    nc.gpsimd.dma_start(vt[:, :blk, :], src)
                else:
                    vt = vpool.tile([P, max_q, L], v_dt, tag="v", name="vt")
                    nc.sync.dma_start(vt[:, :blk, :], src)
                for c in range(blk):
                    rhs = vt[:, c, :]
                    for h in range(2):
                        nc.tensor.matmul(
                            g_ps[h][:],
                            vt[:, c, h * P : (h + 1) * P],
                            rhs,
                            start=(k == 0),
                            stop=(k == CHUNKS - 1),
                        )
                    k += 1
                r0 += P * blk

            o_tile = opool.tile([P, 2, L], f32, tag="o")
            for h in range(2):
                nc.vector.tensor_copy(o_tile[:, h, :], g_ps[h][:])
            nc.sync.dma_start(out, o_tile[:])

    nc.compile()
    _CACHE[mode] = nc
    return nc


def _run(luts, W, trace=False, mode=None, **trace_kwargs):
    """Shard, run on 8 cores, return (loss_scalar, BassKernelResults)."""
    _seed_ntff_hook()
    from concourse.bass_utils import run_bass_kernel_spmd

    nc = _build(mode)

    luts = np.ascontiguousarray(np.asarray(luts, dtype=np.float32))
    W = np.asarray(W, dtype=np.float32)

    in_maps = [{"v": luts[i * SHARD : (i + 1) * SHARD]} for i in range(N_CORES)]
    res = run_bass_kernel_spmd(
        nc, in_maps, core_ids=list(range(N_CORES)), trace=trace, **trace_kwargs
    )

    # host epilogue: loss = sum(M * G_total) / NUM_LUTS  (256x256 - trivial)
    Wd = W.astype(np.float64)
    M = np.diag(Wd.sum(axis=1)) - Wd
    G = np.zeros((L, L), dtype=np.float64)
    for r in res.results:
        g = r["out"].astype(np.float64)  # [128, 2, 256]
        G[:P] += g[:, 0, :]
        G[P:] += g[:, 1, :]
    loss = np.asarray((M * G).sum() / NUM_LUTS, dtype=np.float32)
    return loss, res


def kernel(luts, W, gamma=None, **_unused):
    loss, _ = _run(luts, W, trace=False)
    return loss


if __name__ == "__main__":
    rng = np.random.default_rng(0)
    luts = rng.standard_normal((NUM_LUTS, L), dtype=np.float32)
    W = rng.random((L, L), dtype=np.float32)
    W = (W + W.T) / 2
    np.fill_diagonal(W, 0.0)
    print(kernel(luts, W))
